# revision 20
# baseline (speedup 1.0000x reference)
"""BandSplitLinear Trainium2 kernel (host-transposed fp16 I/O, pure matmul).

Strategy (per core, batch-parallel over 8 cores):
  - Fold w_pre @ w_post into one 128x128 matrix per band on the host (no
    nonlinearity between the linears); biases applied host-side.
  - BIN-PACK whole bands into 33 segments of <= 32 band-width each
    (first-fit decreasing). Every band lives wholly inside one segment,
    so the folded weights form a pure BLOCK-DIAGONAL [33] structure of
    128x128 blocks over the feature layout g = u*4 + c (u = position
    inside the segment, c = channel). No straddling, no off-diagonal
    blocks: one matmul stream per segment.
  - Host passes x already cast to fp16, gathered into the packed layout
    [g (128), j (33), T] — exactly the SBUF layout, so loads/stores are
    plain partition-range strided DMAs. On-chip data flow is pure:
    HBM->SBUF loads, fp16 matmuls with fp32 PSUM accumulation,
    PSUM->SBUF cast copies, SBUF->HBM stores in the same layout. Host
    gathers the output back to [C, T, F] fp32.
  - DMA-bound (~18 MB/core at the ~358 GB/s per-NC HBM limit). One
    dma_start fans across all 16 SDMA engines, so traffic is organized
    as FEW, LARGE, 128-partition transfers with multi-KB contiguous
    per-partition runs. Loads ride the SP HWDGE ring, wall + stores
    ride the Act HWDGE ring — two independent FIFO rings that share the
    16 engines at packet granularity, so reads and writes self-balance
    to the HBM roofline. The gpsimd SWDGE path is unused (saves its
    end-of-kernel drain). A dummy 1-col matmul gates PE start until
    load group 1 is resident so the matmul wave runs warm.
"""

import numpy as np

import concourse.tile as tile
from concourse import bacc, mybir
from concourse.bass_utils import run_bass_kernel_spmd


# ---- problem constants (hardcoded per spec) ----
B, C, T, F = 8, 4, 1000, 1025
N_CORES = 8
SEG = 32  # max band width; also the u-capacity of one 128-partition segment
P = 128
T_CHUNKS = [(0, 512), (512, 488)]  # PSUM bank granularity for matmul
LOAD_GROUPS = [1, 2, 3, 4, 4, 5, 5, 4, 2, 2, 1]  # j-segments per load group (33)
STORE_GROUPS = [1, 2, 3, 3, 4, 4, 4, 4, 3, 2, 2, 1]  # j-segs per store group (33)
# store-group ring: 0 = Act (scalar), 1 = SP (sync). Only the last two
# groups ride the SP ring (it is drained of loads by then); earlier
# sync stores would ping-pong the drain between rings.
STORE_ENGS = [0, 0, 0, 0, 0, 0, 0, 0, 0, 0, 1, 1]
WALL_SPLIT_JO = [0, 4, 18, 33]  # wall load split points (jo boundaries)

_F32 = mybir.dt.float32
_F16 = mybir.dt.float16


def _build_bands():
    f, interval = 0, 4
    groups = []
    while f < F:
        end = min(f + interval, F)
        groups.append((f, end))
        f = end
        if interval < 32:
            interval += 1
    return groups  # list of (start, end), disjoint, covering [0, F)


def _pack_bands():
    """First-fit-decreasing pack of band widths into segments of <= SEG.

    Returns (bands, place, nseg): place[k] = (j, u0) puts band k at
    u-offset u0 of segment j.
    """
    bands = _build_bands()
    order = sorted(
        range(len(bands)), key=lambda k: (bands[k][0] - bands[k][1], k)
    )  # descending width, stable
    fill = []  # per segment: used u
    place = {}
    for k in order:
        w = bands[k][1] - bands[k][0]
        for j in range(len(fill)):
            if fill[j] + w <= SEG:
                place[k] = (j, fill[j])
                fill[j] += w
                break
        else:
            place[k] = (len(fill), 0)
            fill.append(w)
    return bands, place, len(fill)


_BANDS, _PLACE, NSEG = _pack_bands()
assert NSEG == 33, NSEG

# f_of[j, u] = frequency bin occupying slot (j, u), or -1 for pad
_F_OF = np.full((NSEG, SEG), -1, dtype=np.int64)
for _k, (_s, _e) in enumerate(_BANDS):
    _j, _u0 = _PLACE[_k]
    _F_OF[_j, _u0 : _u0 + (_e - _s)] = np.arange(_s, _e)


def _build_weight_blocks(w_pre, w_post):
    """Host: fold per-band linears into one 128x128 block per segment."""
    wc = np.einsum(
        "kio,kod->kid", w_pre.astype(np.float64), w_post.astype(np.float64)
    )  # [45, 128, 128], both feature dims indexed by w*4 + c
    blocks = np.zeros((NSEG, P, P), dtype=np.float64)
    for k, (start, end) in enumerate(_BANDS):
        w = end - start
        j, u0 = _PLACE[k]
        g = (np.arange(w)[:, None] + u0) * C + np.arange(C)[None, :]  # [w, C]
        g = g.reshape(-1)  # packed feature indices of this band in seg j
        src = np.arange(w)[:, None] * C + np.arange(C)[None, :]
        src = src.reshape(-1)
        blocks[j][np.ix_(g, g)] = wc[k][np.ix_(src, src)]
    wall = np.concatenate(list(blocks), axis=1).astype(np.float16)  # [128, 33*128]
    return wall


def _bias_field(b_pre, w_post, b_post):
    """bias[c, f]: the constant added to out[., c, ., f]."""
    bc = (
        np.einsum("ko,kod->kd", b_pre.astype(np.float64), w_post.astype(np.float64))
        + b_post.astype(np.float64)
    )
    field = np.zeros((C, F), dtype=np.float64)
    for k, (start, end) in enumerate(_BANDS):
        for c in range(C):
            field[c, start:end] = bc[k, (np.arange(end - start)) * C + c]
    return field.astype(np.float32)


def _build_nc():
    total_cols = NSEG * P
    nc = bacc.Bacc("TRN2", target_bir_lowering=False, debug=False)
    xs = nc.dram_tensor("xs", [P, NSEG, T], _F16, kind="ExternalInput")
    wall = nc.dram_tensor("wall", [P, total_cols], _F16, kind="ExternalInput")
    ys = nc.dram_tensor("ys", [P, NSEG, T], _F16, kind="ExternalOutput")

    wall_ranges = [
        (lo * P, hi * P) for lo, hi in zip(WALL_SPLIT_JO, WALL_SPLIT_JO[1:])
    ]

    with tile.TileContext(nc) as tc:
        import contextlib

        ctx = contextlib.ExitStack()
        with ctx:
            wall_pool = ctx.enter_context(tc.tile_pool(name="wall", bufs=1))
            at_pools = [
                ctx.enter_context(tc.tile_pool(name=f"atg{i}", bufs=1))
                for i in range(len(LOAD_GROUPS))
            ]
            y_pools = [
                ctx.enter_context(tc.tile_pool(name=f"yg{i}", bufs=1))
                for i in range(len(STORE_GROUPS))
            ]
            psy_pool = ctx.enter_context(
                tc.tile_pool(name="psy", bufs=8, space="PSUM")
            )

            # ---- input tiles: [g = u*4+c, j*T + t] per group ----
            at_tiles = []  # (j0, tile) per group
            j0 = 0
            for gi, gn in enumerate(LOAD_GROUPS):
                at_tiles.append(
                    (j0, at_pools[gi].tile([P, gn * T], _F16, name=f"atg{gi}"))
                )
                j0 += gn

            # One SBUF tile per wall range: Tile dependencies are
            # tile-granular, so a single wall tile would make EVERY
            # matmul wait for the LAST wall DMA. Separate tiles let the
            # jo<4 matmuls start as soon as the small first range lands.
            wall_tiles = [
                wall_pool.tile([P, hi - lo], _F16, name=f"wall{i}")
                for i, (lo, hi) in enumerate(wall_ranges)
            ]

            def wall_block(jo):
                for (lo, hi), wt in zip(wall_ranges, wall_tiles):
                    if lo <= jo * P < hi:
                        return wt[:, jo * P - lo : (jo + 1) * P - lo]
                raise AssertionError(jo)

            def load_wall(eng, i):
                lo, hi = wall_ranges[i]
                eng.dma_start(wall_tiles[i], wall.ap()[:, lo:hi])

            def load_group(eng, gi):
                j0, at_t = at_tiles[gi]
                gn = LOAD_GROUPS[gi]
                eng.dma_start(
                    at_t.rearrange("p (j t) -> p j t", j=gn),
                    xs.ap()[:, j0 : j0 + gn, :],
                )

            # Startup split across both HWDGE rings so descriptor
            # generation and first bytes overlap. SP ring: small wall A
            # (jo 0-3), load group 0, wall B (jo 4-17), then the big
            # loads. Act ring: load group 1, wall C, then the stores.
            load_wall(nc.sync, 0)
            load_group(nc.sync, 0)
            load_group(nc.scalar, 1)
            load_wall(nc.sync, 1)
            load_wall(nc.scalar, 2)
            for gi in range(2, len(LOAD_GROUPS)):
                load_group(nc.sync, gi)

            def at_slice(ji, t0, tn):
                for gi, gn in enumerate(LOAD_GROUPS):
                    j0, at_t = at_tiles[gi]
                    if j0 <= ji < j0 + gn:
                        return at_t[:, (ji - j0) * T + t0 : (ji - j0) * T + t0 + tn]
                raise AssertionError(ji)

            # ---- y staging tiles per store group ----
            y_tiles = []
            j0 = 0
            for gi, gn in enumerate(STORE_GROUPS):
                y_tiles.append(
                    (j0, y_pools[gi].tile([P, gn * T], _F16, name=f"yg{gi}"))
                )
                j0 += gn

            # ---- matmul wavefront over jo, PSUM -> y copies, group stores ----
            gi_store = 0
            for jo in range(NSEG):
                yj0, y_t = y_tiles[gi_store]
                for t0, tn in T_CHUNKS:
                    # one single-bank PSUM tile per chunk: one writer
                    # (matmul) + one reader (copy), 8 bufs deep, so the
                    # PE stream never blocks on a copy.
                    psy = psy_pool.tile([P, 512], _F32, name="psy")
                    nc.tensor.matmul(
                        psy[:, 0:tn],
                        lhsT=wall_block(jo),
                        rhs=at_slice(jo, t0, tn),
                        start=True,
                        stop=True,
                    )
                    dst = y_t[:, (jo - yj0) * T + t0 : (jo - yj0) * T + t0 + tn]
                    # chunk 0 on ACT, chunk 1 on DVE: both chunks of a
                    # segment copy in parallel, halving the PSUM->SBUF
                    # latency on the critical store path
                    if t0 == 0:
                        nc.scalar.copy(dst, psy[:, 0:tn])
                    else:
                        nc.vector.tensor_copy(dst, psy[:, 0:tn])

                # group finished -> store it as one full-width transfer
                gn = STORE_GROUPS[gi_store]
                if jo == yj0 + gn - 1:
                    eng = nc.sync if STORE_ENGS[gi_store] else nc.scalar
                    eng.dma_start(
                        ys.ap()[:, yj0 : yj0 + gn, :],
                        y_t.rearrange("p (j t) -> p j t", j=gn),
                    )
                    gi_store += 1
    nc.compile()
    return nc


_CACHE = {}


def build_in_maps(x, wall):
    """Host prep: wall is the flat [g_in, 33*128] block-diagonal matrix; x
    is cast fp16 and gathered to the packed layout [g = u*4+c (128),
    j(33), T] so each SBUF partition reads one contiguous DRAM run."""
    wall2 = np.ascontiguousarray(wall)
    x16 = np.asarray(x, np.float32).astype(np.float16)  # [B,C,T,F]
    xf = np.concatenate([x16, np.zeros((B, C, T, 1), np.float16)], axis=-1)
    fidx = np.where(_F_OF >= 0, _F_OF, F)  # pad slots read the zero column
    xg = xf[:, :, :, fidx]  # [B, C, T, J, U]
    xp = np.ascontiguousarray(
        xg.transpose(0, 4, 1, 3, 2).reshape(B, P, NSEG, T)
    )
    return [{"xs": xp[b], "wall": wall2} for b in range(N_CORES)]


def kernel(x, w_pre, b_pre, w_post, b_post):
    x = np.asarray(x, dtype=np.float32)
    w_pre = np.asarray(w_pre, dtype=np.float32)
    b_pre = np.asarray(b_pre, dtype=np.float32)
    w_post = np.asarray(w_post, dtype=np.float32)
    b_post = np.asarray(b_post, dtype=np.float32)

    wall = _build_weight_blocks(w_pre, w_post)

    if "nc" not in _CACHE:
        _CACHE["nc"] = _build_nc()
    nc = _CACHE["nc"]

    in_maps = build_in_maps(x, wall)
    res = run_bass_kernel_spmd(nc, in_maps, core_ids=list(range(N_CORES)))
    yp = np.stack([res.results[b]["ys"] for b in range(N_CORES)])  # [B,g,j,T]
    ypt = (
        yp.reshape(B, SEG, C, NSEG, T)
        .transpose(0, 2, 4, 3, 1)  # [B, C, T, J, U]
        .reshape(B, C, T, NSEG * SEG)
    )
    pos = np.zeros(F, dtype=np.int64)
    jj, uu = np.nonzero(_F_OF >= 0)
    pos[_F_OF[jj, uu]] = jj * SEG + uu
    out = ypt[:, :, :, pos].astype(np.float32)

    if np.any(b_pre) or np.any(b_post):
        field = _bias_field(b_pre, w_post, b_post)
        out = out + field[None, :, None, :]
    return np.ascontiguousarray(out)


# revision 21
# speedup vs baseline: 1.0331x; 1.0331x over previous
"""BandSplitLinear Trainium2 kernel (host-transposed fp16 I/O, pure matmul).

Strategy (per core, batch-parallel over 8 cores):
  - Fold w_pre @ w_post into one 128x128 matrix per band on the host (no
    nonlinearity between the linears); biases applied host-side.
  - BIN-PACK whole bands into 33 segments of <= 32 band-width each
    (first-fit decreasing). Every band lives wholly inside one segment,
    so the folded weights form a pure BLOCK-DIAGONAL [33] structure of
    128x128 blocks over the feature layout g = u*4 + c (u = position
    inside the segment, c = channel). No straddling, no off-diagonal
    blocks: one matmul stream per segment.
  - Host passes x already cast to fp16, gathered into the packed layout
    [g (128), j (33), T] — exactly the SBUF layout, so loads/stores are
    plain partition-range strided DMAs. On-chip data flow is pure:
    HBM->SBUF loads, fp16 matmuls with fp32 PSUM accumulation,
    PSUM->SBUF cast copies, SBUF->HBM stores in the same layout. Host
    gathers the output back to [C, T, F] fp32.
  - DMA-bound (~18 MB/core at the ~358 GB/s per-NC HBM limit). One
    dma_start fans across all 16 SDMA engines, so traffic is organized
    as FEW, LARGE, 128-partition transfers with multi-KB contiguous
    per-partition runs. Loads ride the SP HWDGE ring, wall + stores
    ride the Act HWDGE ring — two independent FIFO rings that share the
    16 engines at packet granularity, so reads and writes self-balance
    to the HBM roofline. The gpsimd SWDGE path is unused (saves its
    end-of-kernel drain). A dummy 1-col matmul gates PE start until
    load group 1 is resident so the matmul wave runs warm.
"""

import numpy as np

import concourse.tile as tile
from concourse import bacc, mybir
from concourse.bass_utils import run_bass_kernel_spmd


# ---- problem constants (hardcoded per spec) ----
B, C, T, F = 8, 4, 1000, 1025
N_CORES = 8
SEG = 32  # max band width; also the u-capacity of one 128-partition segment
P = 128
T_CHUNKS = [(0, 512), (512, 488)]  # PSUM bank granularity for matmul
LOAD_GROUPS = [1, 2, 3, 4, 4, 5, 5, 4, 2, 2, 1]  # j-segments per load group (33)
STORE_GROUPS = [1, 2, 3, 3, 4, 4, 4, 4, 3, 2, 2, 1]  # j-segs per store group (33)
# store-group ring: 0 = Act (scalar), 1 = SP (sync). Only the last two
# groups ride the SP ring (it is drained of loads by then); earlier
# sync stores would ping-pong the drain between rings.
STORE_ENGS = [0, 0, 0, 0, 0, 0, 0, 0, 0, 0, 1, 1]
WALL_SPLIT_JO = [0, 4, 18, 33]  # wall load split points (jo boundaries)

_F32 = mybir.dt.float32
_F16 = mybir.dt.float16


def _build_bands():
    f, interval = 0, 4
    groups = []
    while f < F:
        end = min(f + interval, F)
        groups.append((f, end))
        f = end
        if interval < 32:
            interval += 1
    return groups  # list of (start, end), disjoint, covering [0, F)


def _pack_bands():
    """First-fit-decreasing pack of band widths into segments of <= SEG.

    Returns (bands, place, nseg): place[k] = (j, u0) puts band k at
    u-offset u0 of segment j.
    """
    bands = _build_bands()
    order = sorted(
        range(len(bands)), key=lambda k: (bands[k][0] - bands[k][1], k)
    )  # descending width, stable
    fill = []  # per segment: used u
    place = {}
    for k in order:
        w = bands[k][1] - bands[k][0]
        for j in range(len(fill)):
            if fill[j] + w <= SEG:
                place[k] = (j, fill[j])
                fill[j] += w
                break
        else:
            place[k] = (len(fill), 0)
            fill.append(w)
    return bands, place, len(fill)


_BANDS, _PLACE, NSEG = _pack_bands()
assert NSEG == 33, NSEG

# f_of[j, u] = frequency bin occupying slot (j, u), or -1 for pad
_F_OF = np.full((NSEG, SEG), -1, dtype=np.int64)
for _k, (_s, _e) in enumerate(_BANDS):
    _j, _u0 = _PLACE[_k]
    _F_OF[_j, _u0 : _u0 + (_e - _s)] = np.arange(_s, _e)


def _build_weight_blocks(w_pre, w_post):
    """Host: fold per-band linears into one 128x128 block per segment."""
    wc = np.einsum(
        "kio,kod->kid", w_pre.astype(np.float64), w_post.astype(np.float64)
    )  # [45, 128, 128], both feature dims indexed by w*4 + c
    blocks = np.zeros((NSEG, P, P), dtype=np.float64)
    for k, (start, end) in enumerate(_BANDS):
        w = end - start
        j, u0 = _PLACE[k]
        g = (np.arange(w)[:, None] + u0) * C + np.arange(C)[None, :]  # [w, C]
        g = g.reshape(-1)  # packed feature indices of this band in seg j
        src = np.arange(w)[:, None] * C + np.arange(C)[None, :]
        src = src.reshape(-1)
        blocks[j][np.ix_(g, g)] = wc[k][np.ix_(src, src)]
    wall = np.concatenate(list(blocks), axis=1).astype(np.float16)  # [128, 33*128]
    return wall


def _bias_field(b_pre, w_post, b_post):
    """bias[c, f]: the constant added to out[., c, ., f]."""
    bc = (
        np.einsum("ko,kod->kd", b_pre.astype(np.float64), w_post.astype(np.float64))
        + b_post.astype(np.float64)
    )
    field = np.zeros((C, F), dtype=np.float64)
    for k, (start, end) in enumerate(_BANDS):
        for c in range(C):
            field[c, start:end] = bc[k, (np.arange(end - start)) * C + c]
    return field.astype(np.float32)


def _build_nc():
    total_cols = NSEG * P
    nc = bacc.Bacc("TRN2", target_bir_lowering=False, debug=False)
    xs = nc.dram_tensor("xs", [P, NSEG, T], _F16, kind="ExternalInput")
    wall = nc.dram_tensor("wall", [P, total_cols], _F16, kind="ExternalInput")
    ys = nc.dram_tensor("ys", [P, NSEG, T], _F16, kind="ExternalOutput")

    wall_ranges = [
        (lo * P, hi * P) for lo, hi in zip(WALL_SPLIT_JO, WALL_SPLIT_JO[1:])
    ]

    with tile.TileContext(nc) as tc:
        import contextlib

        ctx = contextlib.ExitStack()
        with ctx:
            wall_pool = ctx.enter_context(tc.tile_pool(name="wall", bufs=1))
            at_pools = [
                ctx.enter_context(tc.tile_pool(name=f"atg{i}", bufs=1))
                for i in range(len(LOAD_GROUPS))
            ]
            y_pools = [
                ctx.enter_context(tc.tile_pool(name=f"yg{i}", bufs=1))
                for i in range(len(STORE_GROUPS))
            ]
            psy_pool = ctx.enter_context(
                tc.tile_pool(name="psy", bufs=8, space="PSUM")
            )

            # ---- input tiles: [g = u*4+c, j*T + t] per group ----
            at_tiles = []  # (j0, tile) per group
            j0 = 0
            for gi, gn in enumerate(LOAD_GROUPS):
                at_tiles.append(
                    (j0, at_pools[gi].tile([P, gn * T], _F16, name=f"atg{gi}"))
                )
                j0 += gn

            # One SBUF tile per wall range: Tile dependencies are
            # tile-granular, so a single wall tile would make EVERY
            # matmul wait for the LAST wall DMA. Separate tiles let the
            # jo<4 matmuls start as soon as the small first range lands.
            wall_tiles = [
                wall_pool.tile([P, hi - lo], _F16, name=f"wall{i}")
                for i, (lo, hi) in enumerate(wall_ranges)
            ]

            def wall_block(jo):
                for (lo, hi), wt in zip(wall_ranges, wall_tiles):
                    if lo <= jo * P < hi:
                        return wt[:, jo * P - lo : (jo + 1) * P - lo]
                raise AssertionError(jo)

            def load_wall(eng, i):
                lo, hi = wall_ranges[i]
                eng.dma_start(wall_tiles[i], wall.ap()[:, lo:hi])

            def load_group(eng, gi):
                j0, at_t = at_tiles[gi]
                gn = LOAD_GROUPS[gi]
                eng.dma_start(
                    at_t.rearrange("p (j t) -> p j t", j=gn),
                    xs.ap()[:, j0 : j0 + gn, :],
                )

            # Startup split across both HWDGE rings so descriptor
            # generation and first bytes overlap. SP ring: small wall A
            # (jo 0-3), load group 0, wall B (jo 4-17), then the big
            # loads. Act ring: load group 1, wall C, then the stores.
            load_wall(nc.sync, 0)
            load_group(nc.sync, 0)
            load_group(nc.scalar, 1)
            load_wall(nc.sync, 1)
            load_wall(nc.scalar, 2)
            for gi in range(2, len(LOAD_GROUPS)):
                load_group(nc.sync, gi)

            def at_slice(ji, t0, tn):
                for gi, gn in enumerate(LOAD_GROUPS):
                    j0, at_t = at_tiles[gi]
                    if j0 <= ji < j0 + gn:
                        return at_t[:, (ji - j0) * T + t0 : (ji - j0) * T + t0 + tn]
                raise AssertionError(ji)

            # ---- y staging tiles per store group ----
            y_tiles = []
            j0 = 0
            for gi, gn in enumerate(STORE_GROUPS):
                y_tiles.append(
                    (j0, y_pools[gi].tile([P, gn * T], _F16, name=f"yg{gi}"))
                )
                j0 += gn

            # ---- matmul wavefront over jo, PSUM -> y copies, group stores ----
            gi_store = 0
            for jo in range(NSEG):
                yj0, y_t = y_tiles[gi_store]
                for t0, tn in T_CHUNKS:
                    # one single-bank PSUM tile per chunk: one writer
                    # (matmul) + one reader (copy), 8 bufs deep, so the
                    # PE stream never blocks on a copy.
                    psy = psy_pool.tile([P, 512], _F32, name="psy")
                    if 14 <= jo <= 28 and t0 == 0:
                        # HAM warm-keeper: a throwaway matmul on resident
                        # data fills part of the PE's load-wait gap so the
                        # activity monitor holds the 2.4 GHz clock into
                        # the post-load blast. Output lands in the same
                        # tile and is fully overwritten by the real
                        # start=True matmul below.
                        nc.tensor.matmul(
                            psy[:, 0:tn],
                            lhsT=wall_block(0),
                            rhs=at_slice(0, t0, tn),
                            start=True,
                            stop=True,
                        )
                    nc.tensor.matmul(
                        psy[:, 0:tn],
                        lhsT=wall_block(jo),
                        rhs=at_slice(jo, t0, tn),
                        start=True,
                        stop=True,
                    )
                    dst = y_t[:, (jo - yj0) * T + t0 : (jo - yj0) * T + t0 + tn]
                    # chunk 0 on ACT, chunk 1 on DVE: both chunks of a
                    # segment copy in parallel, halving the PSUM->SBUF
                    # latency on the critical store path
                    if t0 == 0:
                        nc.scalar.copy(dst, psy[:, 0:tn])
                    else:
                        nc.vector.tensor_copy(dst, psy[:, 0:tn])

                # group finished -> store it as one full-width transfer
                gn = STORE_GROUPS[gi_store]
                if jo == yj0 + gn - 1:
                    eng = nc.sync if STORE_ENGS[gi_store] else nc.scalar
                    eng.dma_start(
                        ys.ap()[:, yj0 : yj0 + gn, :],
                        y_t.rearrange("p (j t) -> p j t", j=gn),
                    )
                    gi_store += 1
    nc.compile()
    return nc


_CACHE = {}


def build_in_maps(x, wall):
    """Host prep: wall is the flat [g_in, 33*128] block-diagonal matrix; x
    is cast fp16 and gathered to the packed layout [g = u*4+c (128),
    j(33), T] so each SBUF partition reads one contiguous DRAM run."""
    wall2 = np.ascontiguousarray(wall)
    x16 = np.asarray(x, np.float32).astype(np.float16)  # [B,C,T,F]
    xf = np.concatenate([x16, np.zeros((B, C, T, 1), np.float16)], axis=-1)
    fidx = np.where(_F_OF >= 0, _F_OF, F)  # pad slots read the zero column
    xg = xf[:, :, :, fidx]  # [B, C, T, J, U]
    xp = np.ascontiguousarray(
        xg.transpose(0, 4, 1, 3, 2).reshape(B, P, NSEG, T)
    )
    return [{"xs": xp[b], "wall": wall2} for b in range(N_CORES)]


def kernel(x, w_pre, b_pre, w_post, b_post):
    x = np.asarray(x, dtype=np.float32)
    w_pre = np.asarray(w_pre, dtype=np.float32)
    b_pre = np.asarray(b_pre, dtype=np.float32)
    w_post = np.asarray(w_post, dtype=np.float32)
    b_post = np.asarray(b_post, dtype=np.float32)

    wall = _build_weight_blocks(w_pre, w_post)

    if "nc" not in _CACHE:
        _CACHE["nc"] = _build_nc()
    nc = _CACHE["nc"]

    in_maps = build_in_maps(x, wall)
    res = run_bass_kernel_spmd(nc, in_maps, core_ids=list(range(N_CORES)))
    yp = np.stack([res.results[b]["ys"] for b in range(N_CORES)])  # [B,g,j,T]
    ypt = (
        yp.reshape(B, SEG, C, NSEG, T)
        .transpose(0, 2, 4, 3, 1)  # [B, C, T, J, U]
        .reshape(B, C, T, NSEG * SEG)
    )
    pos = np.zeros(F, dtype=np.int64)
    jj, uu = np.nonzero(_F_OF >= 0)
    pos[_F_OF[jj, uu]] = jj * SEG + uu
    out = ypt[:, :, :, pos].astype(np.float32)

    if np.any(b_pre) or np.any(b_post):
        field = _bias_field(b_pre, w_post, b_post)
        out = out + field[None, :, None, :]
    return np.ascontiguousarray(out)


# revision 28
# speedup vs baseline: 1.0496x; 1.0160x over previous
"""BandSplitLinear Trainium2 kernel (host-transposed fp16 I/O, pure matmul).

Strategy (per core, batch-parallel over 8 cores):
  - Fold w_pre @ w_post into one 128x128 matrix per band on the host (no
    nonlinearity between the linears); biases applied host-side.
  - BIN-PACK whole bands into 33 segments of <= 32 band-width each
    (first-fit decreasing). Every band lives wholly inside one segment,
    so the folded weights form a pure BLOCK-DIAGONAL [33] structure of
    128x128 blocks over the feature layout g = u*4 + c (u = position
    inside the segment, c = channel). No straddling, no off-diagonal
    blocks: one matmul stream per segment.
  - Host passes x already cast to fp16, gathered into the packed layout
    [g (128), j (33), T] — exactly the SBUF layout, so loads/stores are
    plain partition-range strided DMAs. On-chip data flow is pure:
    HBM->SBUF loads, fp16 matmuls with fp32 PSUM accumulation,
    PSUM->SBUF cast copies, SBUF->HBM stores in the same layout. Host
    gathers the output back to [C, T, F] fp32.
  - DMA-bound (~18 MB/core at the ~358 GB/s per-NC HBM limit). One
    dma_start fans across all 16 SDMA engines, so traffic is organized
    as FEW, LARGE, 128-partition transfers with multi-KB contiguous
    per-partition runs. Loads ride the SP HWDGE ring, wall + stores
    ride the Act HWDGE ring — two independent FIFO rings that share the
    16 engines at packet granularity, so reads and writes self-balance
    to the HBM roofline. The gpsimd SWDGE path is unused (saves its
    end-of-kernel drain). A dummy 1-col matmul gates PE start until
    load group 1 is resident so the matmul wave runs warm.
"""

import numpy as np

import concourse.tile as tile
from concourse import bacc, mybir
from concourse.bass_utils import run_bass_kernel_spmd


# ---- problem constants (hardcoded per spec) ----
B, C, T, F = 8, 4, 1000, 1025
N_CORES = 8
SEG = 32  # max band width; also the u-capacity of one 128-partition segment
P = 128
T_CHUNKS = [(0, 512), (512, 488)]  # PSUM bank granularity for matmul
LOAD_GROUPS = [1, 2, 3, 4, 4, 5, 5, 4, 2, 2, 1]  # j-segments per load group (33)
STORE_GROUPS = [1, 2, 3, 3, 4, 4, 4, 4, 3, 2, 2, 1]  # j-segs per store group (33)
# store-group ring: 0 = Act (scalar), 1 = SP (sync). Only the last two
# groups ride the SP ring (it is drained of loads by then); earlier
# sync stores would ping-pong the drain between rings.
STORE_ENGS = [0, 0, 0, 0, 0, 0, 0, 0, 0, 0, 1, 1]
WALL_SPLIT_JO = [0, 4, 18, 33]  # wall load split points (jo boundaries)

_F32 = mybir.dt.float32
_F16 = mybir.dt.float16


def _build_bands():
    f, interval = 0, 4
    groups = []
    while f < F:
        end = min(f + interval, F)
        groups.append((f, end))
        f = end
        if interval < 32:
            interval += 1
    return groups  # list of (start, end), disjoint, covering [0, F)


def _pack_bands():
    """First-fit-decreasing pack of band widths into segments of <= SEG.

    Returns (bands, place, nseg): place[k] = (j, u0) puts band k at
    u-offset u0 of segment j.
    """
    bands = _build_bands()
    order = sorted(
        range(len(bands)), key=lambda k: (bands[k][0] - bands[k][1], k)
    )  # descending width, stable
    fill = []  # per segment: used u
    place = {}
    for k in order:
        w = bands[k][1] - bands[k][0]
        for j in range(len(fill)):
            if fill[j] + w <= SEG:
                place[k] = (j, fill[j])
                fill[j] += w
                break
        else:
            place[k] = (len(fill), 0)
            fill.append(w)
    return bands, place, len(fill)


_BANDS, _PLACE, NSEG = _pack_bands()
assert NSEG == 33, NSEG

# f_of[j, u] = frequency bin occupying slot (j, u), or -1 for pad
_F_OF = np.full((NSEG, SEG), -1, dtype=np.int64)
for _k, (_s, _e) in enumerate(_BANDS):
    _j, _u0 = _PLACE[_k]
    _F_OF[_j, _u0 : _u0 + (_e - _s)] = np.arange(_s, _e)


def _build_weight_blocks(w_pre, w_post):
    """Host: fold per-band linears into one 128x128 block per segment."""
    wc = np.einsum(
        "kio,kod->kid", w_pre.astype(np.float64), w_post.astype(np.float64)
    )  # [45, 128, 128], both feature dims indexed by w*4 + c
    blocks = np.zeros((NSEG, P, P), dtype=np.float64)
    for k, (start, end) in enumerate(_BANDS):
        w = end - start
        j, u0 = _PLACE[k]
        g = (np.arange(w)[:, None] + u0) * C + np.arange(C)[None, :]  # [w, C]
        g = g.reshape(-1)  # packed feature indices of this band in seg j
        src = np.arange(w)[:, None] * C + np.arange(C)[None, :]
        src = src.reshape(-1)
        blocks[j][np.ix_(g, g)] = wc[k][np.ix_(src, src)]
    wall = np.concatenate(list(blocks), axis=1).astype(np.float16)  # [128, 33*128]
    return wall


def _bias_field(b_pre, w_post, b_post):
    """bias[c, f]: the constant added to out[., c, ., f]."""
    bc = (
        np.einsum("ko,kod->kd", b_pre.astype(np.float64), w_post.astype(np.float64))
        + b_post.astype(np.float64)
    )
    field = np.zeros((C, F), dtype=np.float64)
    for k, (start, end) in enumerate(_BANDS):
        for c in range(C):
            field[c, start:end] = bc[k, (np.arange(end - start)) * C + c]
    return field.astype(np.float32)


def _build_nc(warm_keeper=True, tail_opt=False):
    total_cols = NSEG * P
    nc = bacc.Bacc("TRN2", target_bir_lowering=False, debug=False)
    xs = nc.dram_tensor("xs", [P, NSEG, T], _F16, kind="ExternalInput")
    wall = nc.dram_tensor("wall", [P, total_cols], _F16, kind="ExternalInput")
    ys = nc.dram_tensor("ys", [P, NSEG, T], _F16, kind="ExternalOutput")

    wall_ranges = [
        (lo * P, hi * P) for lo, hi in zip(WALL_SPLIT_JO, WALL_SPLIT_JO[1:])
    ]

    with tile.TileContext(nc) as tc:
        import contextlib

        ctx = contextlib.ExitStack()
        with ctx:
            wall_pool = ctx.enter_context(tc.tile_pool(name="wall", bufs=1))
            at_pools = [
                ctx.enter_context(tc.tile_pool(name=f"atg{i}", bufs=1))
                for i in range(len(LOAD_GROUPS))
            ]
            y_pools = [
                ctx.enter_context(tc.tile_pool(name=f"yg{i}", bufs=1))
                for i in range(len(STORE_GROUPS))
            ]
            psy_pool = ctx.enter_context(
                tc.tile_pool(name="psy", bufs=8, space="PSUM")
            )

            # ---- input tiles: [g = u*4+c, j*T + t] per group ----
            at_tiles = []  # (j0, tile) per group
            j0 = 0
            for gi, gn in enumerate(LOAD_GROUPS):
                at_tiles.append(
                    (j0, at_pools[gi].tile([P, gn * T], _F16, name=f"atg{gi}"))
                )
                j0 += gn

            # One SBUF tile per wall range: Tile dependencies are
            # tile-granular, so a single wall tile would make EVERY
            # matmul wait for the LAST wall DMA. Separate tiles let the
            # jo<4 matmuls start as soon as the small first range lands.
            wall_tiles = [
                wall_pool.tile([P, hi - lo], _F16, name=f"wall{i}")
                for i, (lo, hi) in enumerate(wall_ranges)
            ]

            def wall_block(jo):
                for (lo, hi), wt in zip(wall_ranges, wall_tiles):
                    if lo <= jo * P < hi:
                        return wt[:, jo * P - lo : (jo + 1) * P - lo]
                raise AssertionError(jo)

            def load_wall(eng, i):
                lo, hi = wall_ranges[i]
                eng.dma_start(wall_tiles[i], wall.ap()[:, lo:hi])

            def load_group(eng, gi):
                j0, at_t = at_tiles[gi]
                gn = LOAD_GROUPS[gi]
                eng.dma_start(
                    at_t.rearrange("p (j t) -> p j t", j=gn),
                    xs.ap()[:, j0 : j0 + gn, :],
                )

            # Startup split across both HWDGE rings so descriptor
            # generation and first bytes overlap. SP ring: small wall A
            # (jo 0-3), load group 0, wall B (jo 4-17), then the big
            # loads. Act ring: load group 1, wall C, then the stores.
            if tail_opt:
                # prime both rings with one partition-half of load group
                # 0 each: the first matmul's input dependency completes
                # as early as either ring can move 128 KB
                j00, at_t0 = at_tiles[0]
                gn0 = LOAD_GROUPS[0]
                for (p0, p1), eng in [((0, 64), nc.sync), ((64, P), nc.scalar)]:
                    eng.dma_start(
                        at_t0[p0:p1, :].rearrange("p (j t) -> p j t", j=gn0),
                        xs.ap()[p0:p1, j00 : j00 + gn0, :],
                    )
                load_wall(nc.sync, 0)
            else:
                load_wall(nc.sync, 0)
                load_group(nc.sync, 0)
            load_group(nc.scalar, 1)
            load_wall(nc.sync, 1)
            load_wall(nc.scalar, 2)
            for gi in range(2, len(LOAD_GROUPS)):
                load_group(nc.sync, gi)

            def at_slice(ji, t0, tn):
                for gi, gn in enumerate(LOAD_GROUPS):
                    j0, at_t = at_tiles[gi]
                    if j0 <= ji < j0 + gn:
                        return at_t[:, (ji - j0) * T + t0 : (ji - j0) * T + t0 + tn]
                raise AssertionError(ji)

            # ---- y staging tiles per store group ----
            y_tiles = []
            j0 = 0
            for gi, gn in enumerate(STORE_GROUPS):
                y_tiles.append(
                    (j0, y_pools[gi].tile([P, gn * T], _F16, name=f"yg{gi}"))
                )
                j0 += gn
            # tail opt: the final segment stages per-chunk in two
            # separate tiles so each chunk's store depends only on its
            # own copy; the two stores overlap on the two rings
            if tail_opt:
                assert STORE_GROUPS[-1] == 1
                y_tail = [
                    y_pools[-1].tile([P, tn], _F16, name=f"ytail{ci}")
                    for ci, (t0, tn) in enumerate(T_CHUNKS)
                ]

            # ---- matmul wavefront over jo, PSUM -> y copies, group stores ----
            gi_store = 0
            for jo in range(NSEG):
                yj0, y_t = y_tiles[gi_store]
                for t0, tn in T_CHUNKS:
                    # one single-bank PSUM tile per chunk: one writer
                    # (matmul) + one reader (copy), 8 bufs deep, so the
                    # PE stream never blocks on a copy.
                    psy = psy_pool.tile([P, 512], _F32, name="psy")
                    if warm_keeper and 14 <= jo <= 28 and t0 == 0:
                        # HAM warm-keeper: a throwaway matmul on resident
                        # data fills part of the PE's load-wait gap so the
                        # activity monitor holds the 2.4 GHz clock into
                        # the post-load blast. Output lands in the same
                        # tile and is fully overwritten by the real
                        # start=True matmul below.
                        nc.tensor.matmul(
                            psy[:, 0:tn],
                            lhsT=wall_block(0),
                            rhs=at_slice(0, t0, tn),
                            start=True,
                            stop=True,
                        )
                    nc.tensor.matmul(
                        psy[:, 0:tn],
                        lhsT=wall_block(jo),
                        rhs=at_slice(jo, t0, tn),
                        start=True,
                        stop=True,
                    )
                    last_group = tail_opt and gi_store == len(STORE_GROUPS) - 1
                    if last_group:
                        dst = y_tail[0 if t0 == 0 else 1]
                    else:
                        dst = y_t[
                            :, (jo - yj0) * T + t0 : (jo - yj0) * T + t0 + tn
                        ]
                    # chunk 0 on ACT, chunk 1 on DVE: both chunks of a
                    # segment copy in parallel, halving the PSUM->SBUF
                    # latency on the critical store path
                    if t0 == 0:
                        nc.scalar.copy(dst, psy[:, 0:tn])
                    else:
                        nc.vector.tensor_copy(dst, psy[:, 0:tn])
                    if last_group:
                        # fire each chunk's store as soon as its copy is
                        # done, on its own ring
                        eng = nc.scalar if t0 == 0 else nc.sync
                        eng.dma_start(ys.ap()[:, jo, t0 : t0 + tn], dst)

                # group finished -> store it as one full-width transfer
                gn = STORE_GROUPS[gi_store]
                if jo == yj0 + gn - 1:
                    if not (tail_opt and gi_store == len(STORE_GROUPS) - 1):
                        eng = nc.sync if STORE_ENGS[gi_store] else nc.scalar
                        eng.dma_start(
                            ys.ap()[:, yj0 : yj0 + gn, :],
                            y_t.rearrange("p (j t) -> p j t", j=gn),
                        )
                    gi_store += 1
    nc.compile()
    return nc


_CACHE = {}


def build_in_maps(x, wall):
    """Host prep: wall is the flat [g_in, 33*128] block-diagonal matrix; x
    is cast fp16 and gathered to the packed layout [g = u*4+c (128),
    j(33), T] so each SBUF partition reads one contiguous DRAM run."""
    wall2 = np.ascontiguousarray(wall)
    x16 = np.asarray(x, np.float32).astype(np.float16)  # [B,C,T,F]
    xf = np.concatenate([x16, np.zeros((B, C, T, 1), np.float16)], axis=-1)
    fidx = np.where(_F_OF >= 0, _F_OF, F)  # pad slots read the zero column
    xg = xf[:, :, :, fidx]  # [B, C, T, J, U]
    xp = np.ascontiguousarray(
        xg.transpose(0, 4, 1, 3, 2).reshape(B, P, NSEG, T)
    )
    return [{"xs": xp[b], "wall": wall2} for b in range(N_CORES)]


def kernel(x, w_pre, b_pre, w_post, b_post):
    x = np.asarray(x, dtype=np.float32)
    w_pre = np.asarray(w_pre, dtype=np.float32)
    b_pre = np.asarray(b_pre, dtype=np.float32)
    w_post = np.asarray(w_post, dtype=np.float32)
    b_post = np.asarray(b_post, dtype=np.float32)

    wall = _build_weight_blocks(w_pre, w_post)

    if "nc" not in _CACHE:
        _CACHE["nc"] = _build_nc()
    nc = _CACHE["nc"]

    in_maps = build_in_maps(x, wall)
    res = run_bass_kernel_spmd(nc, in_maps, core_ids=list(range(N_CORES)))
    yp = np.stack([res.results[b]["ys"] for b in range(N_CORES)])  # [B,g,j,T]
    ypt = (
        yp.reshape(B, SEG, C, NSEG, T)
        .transpose(0, 2, 4, 3, 1)  # [B, C, T, J, U]
        .reshape(B, C, T, NSEG * SEG)
    )
    pos = np.zeros(F, dtype=np.int64)
    jj, uu = np.nonzero(_F_OF >= 0)
    pos[_F_OF[jj, uu]] = jj * SEG + uu
    out = ypt[:, :, :, pos].astype(np.float32)

    if np.any(b_pre) or np.any(b_post):
        field = _bias_field(b_pre, w_post, b_post)
        out = out + field[None, :, None, :]
    return np.ascontiguousarray(out)
